# revision 24
# baseline (speedup 1.0000x reference)
"""ECE loss kernel for Trainium2, data-parallel over 8 NeuronCores.

Host side shards + permutes samples (the binning is permutation invariant)
into 128-sample single-label "slots" so the device never needs a per-sample
label gather: the accuracy test becomes a strided column read baked into the
access pattern.  Device computes per-sample confidence (no-max-subtraction
softmax is safe for N(0,1) logits), bins via 15 threshold compares, and
accumulates per-bin (sum_conf, sum_acc) with one PE matmul per tile.  The
final ECE is sum_b |sum_conf_b - sum_acc_b| / N, all-reduced across cores.
"""

import dataclasses
import hashlib
import sys

import numpy as np

sys.path.insert(0, "/opt/trn_rl_repo")

from concourse import bacc, bass, mybir, tile  # noqa: E402
from concourse import bass_utils  # noqa: E402

P = 128          # partitions
SPP = 32         # samples per partition per tile (groups/slots per tile)
TILE = P * SPP   # samples per tile
C = 100          # classes
NBINS = 15
N_CORES = 8
BIG = 80.0       # pad-row logit; exp(80) finite in f32, exp(-80) -> 0
N_TOTAL = 2_000_000
K_ACT = 6        # groups/tile whose exp+sum runs fused on ScalarE (rest: DVE)
DMA_PAIR = 2     # logical tiles loaded per dma_start (bigger rows, better BW)

F32 = mybir.dt.float32
AX = mybir.AxisListType
ALU = mybir.AluOpType
ACTF = mybir.ActivationFunctionType


# ---------------------------------------------------------------- host layout

def build_plan(labels: np.ndarray, n_cores: int = N_CORES):
    """Deal samples round-robin per label so every core has the same number
    of 128-sample slots per label.  Returns (slot_labels, per-core sample
    index arrays with -1 for pad rows)."""
    labels = np.asarray(labels).astype(np.int64).ravel()
    order = np.argsort(labels, kind="stable")
    sorted_labels = labels[order]
    # block boundaries per label
    starts = np.searchsorted(sorted_labels, np.arange(C))
    ends = np.searchsorted(sorted_labels, np.arange(C), side="right")

    slot_labels = []
    core_chunks = [[] for _ in range(n_cores)]
    for k in range(C):
        idx_k = order[starts[k]:ends[k]]
        # core c takes idx_k[c::n_cores]
        per_core = [idx_k[c::n_cores] for c in range(n_cores)]
        max_cnt = max(len(x) for x in per_core)
        slots_k = max(1, -(-max_cnt // P)) if max_cnt > 0 else 0
        if slots_k == 0:
            continue
        padded = slots_k * P
        for c in range(n_cores):
            buf = np.full(padded, -1, dtype=np.int64)
            buf[: len(per_core[c])] = per_core[c]
            core_chunks[c].append(buf)
        slot_labels.extend([k] * slots_k)

    n_slots = len(slot_labels)
    # pad slot count to a full DMA-pair multiple (pad slots use label 0)
    pad_slots = (-n_slots) % (SPP * DMA_PAIR)
    if pad_slots:
        for c in range(n_cores):
            core_chunks[c].append(np.full(pad_slots * P, -1, dtype=np.int64))
        slot_labels.extend([0] * pad_slots)
        n_slots += pad_slots

    slot_labels = np.asarray(slot_labels, dtype=np.int64)
    core_idx = [np.concatenate(ch) for ch in core_chunks]
    T = n_slots // SPP
    return slot_labels, core_idx, T


def label_runs(slot_labels: np.ndarray, T: int):
    """Per tile: list of (g0, g1, k) runs of equal-label slots."""
    runs = []
    for t in range(T):
        ks = slot_labels[t * SPP:(t + 1) * SPP]
        tile_runs = []
        g0 = 0
        for g in range(1, SPP + 1):
            if g == SPP or ks[g] != ks[g0]:
                tile_runs.append((g0, g, int(ks[g0])))
                g0 = g
        runs.append(tile_runs)
    return runs


def build_core_slab(logits: np.ndarray, idx: np.ndarray,
                    slot_labels: np.ndarray) -> np.ndarray:
    """Materialize one core's [T*TILE, C] f32 slab in device tile order:
    row (t*TILE + p*SPP + g) holds the p-th sample of slot t*SPP+g."""
    S = len(slot_labels)
    arr = logits[np.maximum(idx, 0)].astype(np.float32, copy=True)
    pad_pos = np.nonzero(idx < 0)[0]
    if len(pad_pos):
        ks = slot_labels[pad_pos // P]
        arr[pad_pos] = -BIG
        arr[pad_pos, ks] = BIG
    # [S, P, C] slot-major -> [Tpair, DMA_PAIR, SPP, P, C] -> pair-DMA order
    # [Tpair, P, DMA_PAIR, SPP, C]: each partition holds DMA_PAIR*SPP
    # consecutive samples of one pair-load.
    arr = arr.reshape(S // (SPP * DMA_PAIR), DMA_PAIR, SPP, P, C)
    arr = arr.transpose(0, 3, 1, 2, 4)
    return np.ascontiguousarray(arr).reshape(-1, C)


# ------------------------------------------------------------- device program

def _bcast(ap, extra):
    """Append a step-0 (broadcast) dim of size `extra` to an AP."""
    return dataclasses.replace(ap, ap=ap.ap + [[0, extra]])


def build_program(T: int, runs, n_total: int, n_cores: int = N_CORES):
    nc = bacc.Bacc("TRN2", target_bir_lowering=False, debug=False,
                   num_devices=n_cores)

    logits_d = nc.dram_tensor("logits", [T * TILE, C], F32, kind="ExternalInput")
    tempr_d = nc.dram_tensor("tempr", [P, 1], F32, kind="ExternalInput")
    thr_d = nc.dram_tensor("thr", [P, SPP * NBINS], F32, kind="ExternalInput")
    wvec_d = nc.dram_tensor("wvec", [2, 1], F32, kind="ExternalInput")
    out_d = nc.dram_tensor("out", [1], F32, kind="ExternalOutput")

    with tile.TileContext(nc) as tc:
        with (
            tc.tile_pool(name="const", bufs=1) as const,
            tc.tile_pool(name="rawp", bufs=4) as rawp,
            tc.tile_pool(name="sb", bufs=3) as sbp,
            tc.tile_pool(name="psH", bufs=1, space="PSUM") as psH,
            tc.tile_pool(name="psF", bufs=1, space="PSUM") as psF,
            tc.tile_pool(name="dram", bufs=1, space="DRAM") as dram,
        ):
            tempr_t = const.tile([P, 1], F32)
            nc.sync.dma_start(tempr_t, tempr_d.ap())
            thr_t = const.tile([P, SPP * NBINS], F32)
            nc.sync.dma_start(thr_t, thr_d.ap())
            wvec_t = const.tile([2, 1], F32)
            nc.sync.dma_start(wvec_t, wvec_d.ap())
            invT = const.tile([P, 1], F32)
            nc.vector.reciprocal(invT, tempr_t)

            hist = psH.tile([2 * SPP, SPP * NBINS], F32)

            assert T % DMA_PAIR == 0
            logits_ap = logits_d.ap()
            for t in range(T):
                h = t % DMA_PAIR
                if h == 0:
                    rawp_t = rawp.tile([P, DMA_PAIR * SPP * C], F32,
                                       tag="raw", name="rawp_t")
                    src = logits_ap[t * TILE:(t + DMA_PAIR) * TILE,
                                    :].rearrange("(p s) c -> p (s c)", p=P)
                    nc.sync.dma_start(rawp_t, src)
                raw = rawp_t[:, h * SPP * C:(h + 1) * SPP * C]

                raw3 = raw.rearrange("p (g c) -> p g c", g=SPP)
                m = sbp.tile([P, SPP], F32, tag="m", name="m")
                nc.vector.reduce_max(m, raw3, axis=AX.X)

                # denominators: ScalarE handles K_ACT groups with fused
                # exp+accum; DVE 3D-sums the rest over one big-FD exp.
                D = sbp.tile([P, SPP], F32, tag="D", name="D")
                for g in range(K_ACT):
                    expg = sbp.tile([P, C], F32, tag="expg", name="expg")
                    nc.scalar.activation(expg, raw[:, g * C:(g + 1) * C],
                                         ACTF.Exp, scale=invT,
                                         accum_out=D[:, g:g + 1])
                if K_ACT < SPP:
                    nd = SPP - K_ACT
                    expA = sbp.tile([P, nd * C], F32, tag="expA", name="expA")
                    nc.scalar.activation(expA, raw[:, K_ACT * C:], ACTF.Exp,
                                         scale=invT)
                    nc.vector.reduce_sum(
                        D[:, K_ACT:SPP],
                        expA.rearrange("p (g c) -> p g c", g=nd), axis=AX.X)

                rd = sbp.tile([P, SPP], F32, tag="rd", name="rd")
                nc.vector.reciprocal(rd, D)
                expm = sbp.tile([P, SPP], F32, tag="expm", name="expm")
                nc.scalar.activation(expm, m, ACTF.Exp, scale=invT)

                pack = sbp.tile([P, 2 * SPP], F32, tag="pack", name="pack")
                nc.vector.tensor_tensor(pack[:, 0:2 * SPP:2], expm, rd,
                                        op=ALU.mult)
                for (g0, g1, k) in runs[t]:
                    lab = raw3[:, g0:g1, k:k + 1].opt()
                    nc.vector.tensor_tensor(
                        pack[:, 2 * g0 + 1:2 * g1:2], lab,
                        m[:, g0:g1], op=ALU.is_ge)

                mask = sbp.tile([P, SPP * NBINS], F32, tag="mask", name="mask")
                conf_b = _bcast(pack[:, 0:2 * SPP:2], NBINS)
                thr3 = thr_t.rearrange("p (g b) -> p g b", g=SPP)
                mask3 = mask.rearrange("p (g b) -> p g b", g=SPP)
                nc.vector.tensor_tensor(mask3, conf_b, thr3, op=ALU.is_gt)

                nc.tensor.matmul(hist, lhsT=pack, rhs=mask,
                                 start=(t == 0), stop=(t == T - 1))

            # ---- finalize: collapse diagonal blocks, cum->bin, allreduce
            hist_sb = sbp.tile([2 * SPP, SPP * NBINS], F32)
            nc.vector.tensor_copy(hist_sb, hist)
            stats3 = sbp.tile([2, SPP * NBINS], F32)
            for q in range(SPP):
                nc.sync.dma_start(
                    stats3[:, q * NBINS:(q + 1) * NBINS],
                    hist_sb[2 * q:2 * q + 2, q * NBINS:(q + 1) * NBINS])
            cum = sbp.tile([2, NBINS], F32)
            nc.vector.reduce_sum(
                cum, stats3.rearrange("p (q b) -> p b q", q=SPP), axis=AX.X)
            cum16 = sbp.tile([2, NBINS + 1], F32)
            nc.vector.memset(cum16, 0.0)
            nc.vector.tensor_copy(cum16[:, 0:NBINS], cum)
            bstats = sbp.tile([2, NBINS], F32)
            nc.vector.tensor_tensor(bstats, cum16[:, 0:NBINS],
                                    cum16[:, 1:NBINS + 1], op=ALU.subtract)

            cc_in = dram.tile([2, NBINS], F32)
            cc_out = dram.tile([2, NBINS], F32)
            nc.sync.dma_start(cc_in, bstats)
            nc.gpsimd.collective_compute(
                "AllReduce", ALU.add,
                replica_groups=[list(range(n_cores))],
                ins=[cc_in.opt()], outs=[cc_out.opt()])
            ar = sbp.tile([2, NBINS], F32)
            nc.sync.dma_start(ar, cc_out)

            dd = psF.tile([1, NBINS], F32)
            nc.tensor.matmul(dd, lhsT=wvec_t, rhs=ar, start=True, stop=True)
            esum = sbp.tile([1, 1], F32)
            nc.vector.tensor_reduce(esum, dd, axis=AX.X, op=ALU.add,
                                    apply_absolute_value=True)
            res = sbp.tile([1, 1], F32)
            nc.scalar.mul(res, esum, 1.0 / n_total)
            nc.sync.dma_start(out_d.ap(), res)

    nc.compile()
    return nc


# ------------------------------------------------------------------- runner

def make_const_inputs():
    thr = np.tile((np.arange(NBINS, dtype=np.float32) / np.float32(NBINS)),
                  SPP)
    return {
        "thr": np.broadcast_to(thr, (P, SPP * NBINS)).copy(),
        "wvec": np.array([[1.0], [-1.0]], np.float32),
    }


_CACHE = {}


def _prepare(logits, labels, temperature, n_total, n_cores=N_CORES):
    labels = np.asarray(labels)
    key = hashlib.sha1(labels.tobytes()).hexdigest()
    if key in _CACHE:
        nc, slot_labels, core_idx, T = _CACHE[key]
    else:
        slot_labels, core_idx, T = build_plan(labels, n_cores)
        nc = build_program(T, label_runs(slot_labels, T), n_total, n_cores)
        _CACHE[key] = (nc, slot_labels, core_idx, T)

    logits = np.asarray(logits, dtype=np.float32)
    consts = make_const_inputs()
    tempr = np.broadcast_to(
        np.asarray(temperature, np.float32).ravel()[0:1], (P, 1)).copy()
    in_maps = []
    for c in range(n_cores):
        m = dict(consts)
        m["tempr"] = tempr
        m["logits"] = build_core_slab(logits, core_idx[c], slot_labels)
        in_maps.append(m)
    return nc, in_maps


def _ensure_ntff_hook():
    """This container's antenv lacks axon_hooks; synthesize it and register
    the ctypes NTFF hook so trace=True works under axon."""
    try:
        import antenv.axon_hooks  # noqa: F401
        return
    except ImportError:
        pass
    import types

    import antenv

    mod = types.ModuleType("antenv.axon_hooks")
    _hook = [None]
    mod.set_axon_ntff_profile_hook = lambda h: _hook.__setitem__(0, h)
    mod.get_axon_ntff_profile_hook = lambda: _hook[0]
    sys.modules["antenv.axon_hooks"] = mod
    antenv.axon_hooks = mod
    try:
        from trn_agent_boot.trn_boot import _ntff_profile_via_ctypes
        mod.set_axon_ntff_profile_hook(
            _ntff_profile_via_ctypes("/opt/axon/libaxon_pjrt.so"))
    except Exception:
        pass


def run(logits, labels, temperature, n_total=None, trace=False,
        n_cores=N_CORES):
    if trace:
        _ensure_ntff_hook()
    if n_total is None:
        n_total = int(np.asarray(labels).shape[0])
    nc, in_maps = _prepare(logits, labels, temperature, n_total, n_cores)
    res = bass_utils.run_bass_kernel_spmd(
        nc, in_maps, core_ids=list(range(n_cores)), trace=trace)
    out = np.asarray(res.results[0]["out"], dtype=np.float32).reshape(1)
    return out, res


def kernel(logits, labels, temperature):
    out, _ = run(logits, labels, temperature)
    return out


# revision 25
# speedup vs baseline: 1.0210x; 1.0210x over previous
"""ECE loss kernel for Trainium2, data-parallel over 8 NeuronCores.

Host side shards + permutes samples (the binning is permutation invariant)
into 128-sample single-label "slots" so the device never needs a per-sample
label gather: the accuracy test becomes a strided column read baked into the
access pattern.  Device computes per-sample confidence (no-max-subtraction
softmax is safe for N(0,1) logits), bins via 15 threshold compares, and
accumulates per-bin (sum_conf, sum_acc) with one PE matmul per tile.  The
final ECE is sum_b |sum_conf_b - sum_acc_b| / N, all-reduced across cores.
"""

import dataclasses
import hashlib
import sys

import numpy as np

sys.path.insert(0, "/opt/trn_rl_repo")

from concourse import bacc, bass, mybir, tile  # noqa: E402
from concourse import bass_utils  # noqa: E402

P = 128          # partitions
SPP = 32         # samples per partition per tile (groups/slots per tile)
TILE = P * SPP   # samples per tile
C = 100          # classes
NBINS = 15
N_CORES = 8
BIG = 80.0       # pad-row logit; exp(80) finite in f32, exp(-80) -> 0
N_TOTAL = 2_000_000
K_ACT = 6        # groups/tile whose exp+sum runs fused on ScalarE (rest: DVE)
DMA_PAIR = 2     # logical tiles loaded per dma_start (bigger rows, better BW)

F32 = mybir.dt.float32
AX = mybir.AxisListType
ALU = mybir.AluOpType
ACTF = mybir.ActivationFunctionType


# ---------------------------------------------------------------- host layout

def build_plan(labels: np.ndarray, n_cores: int = N_CORES):
    """Deal samples round-robin per label so every core has the same number
    of 128-sample slots per label.  Returns (slot_labels, per-core sample
    index arrays with -1 for pad rows)."""
    labels = np.asarray(labels).astype(np.int64).ravel()
    order = np.argsort(labels, kind="stable")
    sorted_labels = labels[order]
    # block boundaries per label
    starts = np.searchsorted(sorted_labels, np.arange(C))
    ends = np.searchsorted(sorted_labels, np.arange(C), side="right")

    slot_labels = []
    core_chunks = [[] for _ in range(n_cores)]
    for k in range(C):
        idx_k = order[starts[k]:ends[k]]
        # core c takes idx_k[c::n_cores]
        per_core = [idx_k[c::n_cores] for c in range(n_cores)]
        max_cnt = max(len(x) for x in per_core)
        slots_k = max(1, -(-max_cnt // P)) if max_cnt > 0 else 0
        if slots_k == 0:
            continue
        padded = slots_k * P
        for c in range(n_cores):
            buf = np.full(padded, -1, dtype=np.int64)
            buf[: len(per_core[c])] = per_core[c]
            core_chunks[c].append(buf)
        slot_labels.extend([k] * slots_k)

    n_slots = len(slot_labels)
    # pad slot count to a full DMA-pair multiple (pad slots use label 0)
    pad_slots = (-n_slots) % (SPP * DMA_PAIR)
    if pad_slots:
        for c in range(n_cores):
            core_chunks[c].append(np.full(pad_slots * P, -1, dtype=np.int64))
        slot_labels.extend([0] * pad_slots)
        n_slots += pad_slots

    slot_labels = np.asarray(slot_labels, dtype=np.int64)
    core_idx = [np.concatenate(ch) for ch in core_chunks]
    T = n_slots // SPP
    return slot_labels, core_idx, T


def label_runs(slot_labels: np.ndarray, T: int):
    """Per tile: list of (g0, g1, k) runs of equal-label slots."""
    runs = []
    for t in range(T):
        ks = slot_labels[t * SPP:(t + 1) * SPP]
        tile_runs = []
        g0 = 0
        for g in range(1, SPP + 1):
            if g == SPP or ks[g] != ks[g0]:
                tile_runs.append((g0, g, int(ks[g0])))
                g0 = g
        runs.append(tile_runs)
    return runs


def build_core_slab(logits: np.ndarray, idx: np.ndarray,
                    slot_labels: np.ndarray) -> np.ndarray:
    """Materialize one core's [T*TILE, C] f32 slab in device tile order:
    row (t*TILE + p*SPP + g) holds the p-th sample of slot t*SPP+g."""
    S = len(slot_labels)
    arr = logits[np.maximum(idx, 0)].astype(np.float32, copy=True)
    pad_pos = np.nonzero(idx < 0)[0]
    if len(pad_pos):
        ks = slot_labels[pad_pos // P]
        arr[pad_pos] = -BIG
        arr[pad_pos, ks] = BIG
    # [S, P, C] slot-major -> [Tpair, DMA_PAIR, SPP, P, C] -> pair-DMA order
    # [Tpair, P, DMA_PAIR, SPP, C]: each partition holds DMA_PAIR*SPP
    # consecutive samples of one pair-load.
    arr = arr.reshape(S // (SPP * DMA_PAIR), DMA_PAIR, SPP, P, C)
    arr = arr.transpose(0, 3, 1, 2, 4)
    return np.ascontiguousarray(arr).reshape(-1, C)


# ------------------------------------------------------------- device program

def _bcast(ap, extra):
    """Append a step-0 (broadcast) dim of size `extra` to an AP."""
    return dataclasses.replace(ap, ap=ap.ap + [[0, extra]])


def build_program(T: int, runs, n_total: int, n_cores: int = N_CORES):
    nc = bacc.Bacc("TRN2", target_bir_lowering=False, debug=False,
                   num_devices=n_cores)

    logits_d = nc.dram_tensor("logits", [T * TILE, C], F32, kind="ExternalInput")
    tempr_d = nc.dram_tensor("tempr", [P, 1], F32, kind="ExternalInput")
    thr_d = nc.dram_tensor("thr", [P, SPP * NBINS], F32, kind="ExternalInput")
    wvec_d = nc.dram_tensor("wvec", [2, 1], F32, kind="ExternalInput")
    out_d = nc.dram_tensor("out", [1], F32, kind="ExternalOutput")

    with tile.TileContext(nc) as tc:
        with (
            tc.tile_pool(name="const", bufs=1) as const,
            tc.tile_pool(name="rawp", bufs=3) as rawp,
            tc.tile_pool(name="sb", bufs=3) as sbp,
            tc.tile_pool(name="psH", bufs=1, space="PSUM") as psH,
            tc.tile_pool(name="psF", bufs=1, space="PSUM") as psF,
            tc.tile_pool(name="dram", bufs=1, space="DRAM") as dram,
        ):
            tempr_t = const.tile([P, 1], F32)
            nc.sync.dma_start(tempr_t, tempr_d.ap())
            thr_t = const.tile([P, SPP * NBINS], F32)
            nc.sync.dma_start(thr_t, thr_d.ap())
            wvec_t = const.tile([2, 1], F32)
            nc.sync.dma_start(wvec_t, wvec_d.ap())
            invT = const.tile([P, 1], F32)
            nc.vector.reciprocal(invT, tempr_t)

            hist = psH.tile([2 * SPP, SPP * NBINS], F32)

            assert T % DMA_PAIR == 0
            logits_ap = logits_d.ap()
            for t in range(T):
                h = t % DMA_PAIR
                if h == 0:
                    rawp_t = rawp.tile([P, DMA_PAIR * SPP * C], F32,
                                       tag="raw", name="rawp_t")
                    src = logits_ap[t * TILE:(t + DMA_PAIR) * TILE,
                                    :].rearrange("(p s) c -> p (s c)", p=P)
                    nc.sync.dma_start(rawp_t, src)
                raw = rawp_t[:, h * SPP * C:(h + 1) * SPP * C]

                raw3 = raw.rearrange("p (g c) -> p g c", g=SPP)
                m = sbp.tile([P, SPP], F32, tag="m", name="m")
                nc.vector.reduce_max(m, raw3, axis=AX.X)

                # denominators: ScalarE handles K_ACT groups with fused
                # exp+accum; DVE 3D-sums the rest over one big-FD exp.
                D = sbp.tile([P, SPP], F32, tag="D", name="D")
                for g in range(K_ACT):
                    expg = sbp.tile([P, C], F32, tag="expg", name="expg")
                    nc.scalar.activation(expg, raw[:, g * C:(g + 1) * C],
                                         ACTF.Exp, scale=invT,
                                         accum_out=D[:, g:g + 1])
                if K_ACT < SPP:
                    nd = SPP - K_ACT
                    expA = sbp.tile([P, nd * C], F32, tag="expA", name="expA")
                    nc.scalar.activation(expA, raw[:, K_ACT * C:], ACTF.Exp,
                                         scale=invT)
                    nc.vector.reduce_sum(
                        D[:, K_ACT:SPP],
                        expA.rearrange("p (g c) -> p g c", g=nd), axis=AX.X)

                rd = sbp.tile([P, SPP], F32, tag="rd", name="rd")
                nc.vector.reciprocal(rd, D)
                expm = sbp.tile([P, SPP], F32, tag="expm", name="expm")
                nc.scalar.activation(expm, m, ACTF.Exp, scale=invT)

                pack = sbp.tile([P, 2 * SPP], F32, tag="pack", name="pack")
                nc.vector.tensor_tensor(pack[:, 0:2 * SPP:2], expm, rd,
                                        op=ALU.mult)
                for (g0, g1, k) in runs[t]:
                    lab = raw3[:, g0:g1, k:k + 1].opt()
                    nc.vector.tensor_tensor(
                        pack[:, 2 * g0 + 1:2 * g1:2], lab,
                        m[:, g0:g1], op=ALU.is_ge)

                mask = sbp.tile([P, SPP * NBINS], F32, tag="mask", name="mask")
                conf_b = _bcast(pack[:, 0:2 * SPP:2], NBINS)
                thr3 = thr_t.rearrange("p (g b) -> p g b", g=SPP)
                mask3 = mask.rearrange("p (g b) -> p g b", g=SPP)
                nc.vector.tensor_tensor(mask3, conf_b, thr3, op=ALU.is_gt)

                nc.tensor.matmul(hist, lhsT=pack, rhs=mask,
                                 start=(t == 0), stop=(t == T - 1))

            # ---- finalize: collapse diagonal blocks, cum->bin, allreduce
            hist_sb = sbp.tile([2 * SPP, SPP * NBINS], F32)
            nc.vector.tensor_copy(hist_sb, hist)
            stats3 = sbp.tile([2, SPP * NBINS], F32)
            for q in range(SPP):
                nc.sync.dma_start(
                    stats3[:, q * NBINS:(q + 1) * NBINS],
                    hist_sb[2 * q:2 * q + 2, q * NBINS:(q + 1) * NBINS])
            cum = sbp.tile([2, NBINS], F32)
            nc.vector.reduce_sum(
                cum, stats3.rearrange("p (q b) -> p b q", q=SPP), axis=AX.X)
            cum16 = sbp.tile([2, NBINS + 1], F32)
            nc.vector.memset(cum16, 0.0)
            nc.vector.tensor_copy(cum16[:, 0:NBINS], cum)
            bstats = sbp.tile([2, NBINS], F32)
            nc.vector.tensor_tensor(bstats, cum16[:, 0:NBINS],
                                    cum16[:, 1:NBINS + 1], op=ALU.subtract)

            cc_in = dram.tile([2, NBINS], F32)
            cc_out = dram.tile([2, NBINS], F32)
            nc.sync.dma_start(cc_in, bstats)
            nc.gpsimd.collective_compute(
                "AllReduce", ALU.add,
                replica_groups=[list(range(n_cores))],
                ins=[cc_in.opt()], outs=[cc_out.opt()])
            ar = sbp.tile([2, NBINS], F32)
            nc.sync.dma_start(ar, cc_out)

            dd = psF.tile([1, NBINS], F32)
            nc.tensor.matmul(dd, lhsT=wvec_t, rhs=ar, start=True, stop=True)
            esum = sbp.tile([1, 1], F32)
            nc.vector.tensor_reduce(esum, dd, axis=AX.X, op=ALU.add,
                                    apply_absolute_value=True)
            res = sbp.tile([1, 1], F32)
            nc.scalar.mul(res, esum, 1.0 / n_total)
            nc.sync.dma_start(out_d.ap(), res)

    nc.compile()
    return nc


# ------------------------------------------------------------------- runner

def make_const_inputs():
    thr = np.tile((np.arange(NBINS, dtype=np.float32) / np.float32(NBINS)),
                  SPP)
    return {
        "thr": np.broadcast_to(thr, (P, SPP * NBINS)).copy(),
        "wvec": np.array([[1.0], [-1.0]], np.float32),
    }


_CACHE = {}


def _prepare(logits, labels, temperature, n_total, n_cores=N_CORES):
    labels = np.asarray(labels)
    key = hashlib.sha1(labels.tobytes()).hexdigest()
    if key in _CACHE:
        nc, slot_labels, core_idx, T = _CACHE[key]
    else:
        slot_labels, core_idx, T = build_plan(labels, n_cores)
        nc = build_program(T, label_runs(slot_labels, T), n_total, n_cores)
        _CACHE[key] = (nc, slot_labels, core_idx, T)

    logits = np.asarray(logits, dtype=np.float32)
    consts = make_const_inputs()
    tempr = np.broadcast_to(
        np.asarray(temperature, np.float32).ravel()[0:1], (P, 1)).copy()
    in_maps = []
    for c in range(n_cores):
        m = dict(consts)
        m["tempr"] = tempr
        m["logits"] = build_core_slab(logits, core_idx[c], slot_labels)
        in_maps.append(m)
    return nc, in_maps


def _ensure_ntff_hook():
    """This container's antenv lacks axon_hooks; synthesize it and register
    the ctypes NTFF hook so trace=True works under axon."""
    try:
        import antenv.axon_hooks  # noqa: F401
        return
    except ImportError:
        pass
    import types

    import antenv

    mod = types.ModuleType("antenv.axon_hooks")
    _hook = [None]
    mod.set_axon_ntff_profile_hook = lambda h: _hook.__setitem__(0, h)
    mod.get_axon_ntff_profile_hook = lambda: _hook[0]
    sys.modules["antenv.axon_hooks"] = mod
    antenv.axon_hooks = mod
    try:
        from trn_agent_boot.trn_boot import _ntff_profile_via_ctypes
        mod.set_axon_ntff_profile_hook(
            _ntff_profile_via_ctypes("/opt/axon/libaxon_pjrt.so"))
    except Exception:
        pass


def run(logits, labels, temperature, n_total=None, trace=False,
        n_cores=N_CORES):
    if trace:
        _ensure_ntff_hook()
    if n_total is None:
        n_total = int(np.asarray(labels).shape[0])
    nc, in_maps = _prepare(logits, labels, temperature, n_total, n_cores)
    res = bass_utils.run_bass_kernel_spmd(
        nc, in_maps, core_ids=list(range(n_cores)), trace=trace)
    out = np.asarray(res.results[0]["out"], dtype=np.float32).reshape(1)
    return out, res


def kernel(logits, labels, temperature):
    out, _ = run(logits, labels, temperature)
    return out
